# revision 64
# baseline (speedup 1.0000x reference)
"""Trainium2 Bass kernel for the CapsuleLayer dynamic-routing module.

Strategy (8 NeuronCores, CHANNEL-parallel over j = in_channels):
  Each core owns JL = 1152/8 = 144 channels and the FULL batch B=256.
  Partition layout: 9 tiles x 128 partitions, p = 8*j16 + i, tile t
  covering local channels jl = 16*t + j16, global j = 144*c + jl.

  - W is j-sharded (369 KB/core instead of 2.95 MB replicated).
  - u_hat never materialized. Per routing iteration:
      A = Wc * exp(b)        (local, one dense bf16 TT)
      s_part[b, nu] = sum_{(i,jl)} A x  -- 18 accumulated PE matmuls,
               k=128 (i,j) partitions, lhsT = x2 column-slices.
      D_part[n] = sum_jl exp(b)  (ones-matmul + t-fold)
      ONE collective per iteration, fp16 wire (|s_part| <= ~200, so
      fp16 is safe with a wide margin): AllReduce [128, 330] with
      Shared-output (the fast HBM-HBM path); s partials in cols 0:320
      with batch b = 128h+q, D partials in cols 320:330. The last
      iteration uses ReduceScatter instead (each core only needs its
      own 16 batch-rows for the output).
      squash on the full batch (replicated; exact reference semantics
      incl. mag over n); sqrt via Ln+Exp(0.5*) so the whole kernel
      runs on ONE pinned activation table (no 1.3us table reloads).
      a-pass: C[(i,jl), nu] = x^T v (18 PE matmuls, k=128 batch);
               z = Wc .* C with the DVE reading C straight from PSUM;
               u-fold tree (DVE bf16 TT adds); i-fold + i-broadcast
               via tiny ones-matmuls (PE); b += a/B.  All local.
  - Iteration 1 uses c == uniform: A := Wc, Dinv := 1/1152.
  - Output: core c's ReduceScatter chunk covers batches 128h+16c+q;
    every core writes its [16, 320] slice; host reassembles.
"""

import numpy as np

B, I, J, N, U = 256, 8, 1152, 10, 16
NU = N * U            # 160
ITERS = 3
NCORES = 8
JL = J // NCORES      # 144 local channels
T = 9                 # tiles of (16 j x 8 i) = 128 partitions
W320 = 2 * NU         # 320 s columns in the AR payload
WID = W320 + N        # 330 with D columns
ACT_TBL_LN_EXP = 6    # act_info.json index of natural_log_exp_and_others

_CACHE = {}


def _build_nc():
    import concourse.bass as bass
    import concourse.bacc as bacc
    import concourse.tile as tile
    from concourse import mybir

    f32 = mybir.dt.float32
    f16 = mybir.dt.float16
    f8 = mybir.dt.float8e4
    bf16 = mybir.dt.bfloat16
    AL = mybir.AluOpType
    AF = mybir.ActivationFunctionType
    AX = mybir.AxisListType

    nc = bacc.Bacc("TRN2", target_bir_lowering=False, debug=False,
                   num_devices=NCORES)
    wc_d = nc.dram_tensor("wc", [128, T * NU], bf16, kind="ExternalInput").ap()
    x2_d = nc.dram_tensor("x2", [128, T * B], bf16, kind="ExternalInput").ap()
    w8_d = nc.dram_tensor("w8", [128, T * NU], f8, kind="ExternalInput").ap()
    x8_d = nc.dram_tensor("x8", [128, T * B], f8, kind="ExternalInput").ap()
    xb_d = nc.dram_tensor("xb", [128, 2 * T * 128], bf16,
                          kind="ExternalInput").ap()
    em_d = nc.dram_tensor("em", [128, 16], bf16, kind="ExternalInput").ap()
    eb_d = nc.dram_tensor("eb", [16, 128], bf16, kind="ExternalInput").ap()
    v_d = nc.dram_tensor("v", [16, W320], f32, kind="ExternalOutput").ap()

    with tile.TileContext(nc) as tc:
        with (
            tc.tile_pool(name="big", bufs=1) as big,
            tc.tile_pool(name="work", bufs=2) as work,
            tc.tile_pool(name="small", bufs=2) as small,
            tc.tile_pool(name="pers", bufs=1) as pers,
            tc.tile_pool(name="ps_s", bufs=1, space="PSUM") as ps_s,
            tc.tile_pool(name="ps_c", bufs=3, space="PSUM") as ps_c,
            tc.tile_pool(name="ps_m", bufs=1, space="PSUM") as ps_m,
            tc.tile_pool(name="dram", bufs=6, space="DRAM") as dram,
        ):
            # Pin the Ln+Exp activation table once; every activation in the
            # kernel (softmax Exp, squash Ln / Exp-half) is served by it, so
            # the compiler's per-function table reloads (1.3us each) vanish.
            nc.scalar.add_instruction(mybir.InstLoadActFuncSet(
                name="pin_tbl", act_func_set_id=ACT_TBL_LN_EXP))

            # ---------------- load inputs ----------------
            # Iteration 0 runs its s-pass entirely from fp8 copies of W and
            # x (half the head-critical DMA bytes; iter-0's s only steers
            # the b/c update, so fp8 error is damped ~10x before it can
            # reach the output). The bf16 tensors and XB stream in behind
            # them and are only needed after AR0.
            W8 = big.tile([128, T, N, U], f8)
            X8 = big.tile([128, T, B], f8)
            w8_v = w8_d.rearrange("p (t n u) -> p t n u", t=T, n=N)
            x8_v = x8_d.rearrange("p (t b) -> p t b", t=T)
            # t0 alone first so the s-pass starts after ~1/9 of the bytes
            for sl in (slice(0, 1), slice(1, 5), slice(5, 9)):
                nc.sync.dma_start(out=W8[:, sl], in_=w8_v[:, sl])
                nc.sync.dma_start(out=X8[:, sl], in_=x8_v[:, sl])
            WC = big.tile([128, T, N, U], bf16)
            X2 = big.tile([128, T, B], bf16)
            nc.sync.dma_start(out=WC, in_=wc_d.rearrange(
                "p (t n u) -> p t n u", t=T, n=N))
            nc.sync.dma_start(out=X2, in_=x2_d.rearrange(
                "p (t b) -> p t b", t=T))
            # XB/EM/EB are only needed in the a-pass (after AR0) -- their
            # DMAs drain during the collective wait.
            XB = big.tile([128, 2, T, 128], bf16)
            nc.sync.dma_start(out=XB, in_=xb_d.rearrange(
                "p (h t m) -> p h t m", h=2, t=T))
            EM = pers.tile([128, 16], bf16)
            nc.sync.dma_start(out=EM, in_=em_d)
            EB = pers.tile([16, 128], bf16)
            nc.sync.dma_start(out=EB, in_=eb_d)

            ONES = pers.tile([128, 128], bf16)
            nc.vector.memset(ONES, 1.0)
            brep = pers.tile([128, T * N], f32)      # b[(t,n)] replicated in i
            nc.vector.memset(brep, 0.0)
            dinv0 = pers.tile([128, N], f32)
            nc.vector.memset(dinv0, 1.0 / J)

            # No PE warm-ups: traces show the HAM boost window is a
            # depleting budget -- dummy warm matmuls spend it on garbage
            # during collective waits instead of the real bursts.

            WCf = WC.rearrange("p t n u -> p (t n u)")

            for it in range(ITERS):
                first = it == 0
                last = it == ITERS - 1

                wid = W320 if first else WID
                wdt = f16
                ar_sb = small.tile([128, wid], wdt, tag="arsb")
                ar_in = dram.tile([128, wid], wdt, tag=f"arin{it}")

                # ------------ c-pass + s-pass, pipelined by t-group ------
                # A is built in 3 t-groups on the DVE; the PE starts the
                # h=0 s-pass matmuls for group g as soon as A[g] lands.
                if first:
                    A = WC
                else:
                    cexp_b = small.tile([128, T * N], bf16, tag="cexpb")
                    nc.scalar.activation(out=cexp_b, in_=brep, func=AF.Exp)
                    # D_part[n]: column sums via ones-matmul (result
                    # replicated over all 128 partitions), then t-fold.
                    psD = ps_m.tile([128, 96], f32, tag="psm")
                    nc.tensor.matmul(psD[:, 0:T * N], lhsT=ONES, rhs=cexp_b,
                                     start=True, stop=True)
                    D10 = small.tile([128, N], f32, tag="d10")
                    nc.vector.tensor_reduce(
                        out=D10,
                        in_=psD[:, 0:T * N].rearrange("q (t n) -> q n t", t=T),
                        axis=AX.X, op=AL.add)
                    # D wire-cast hoisted here: it runs during the s-pass,
                    # off the collective trigger path.
                    nc.scalar.activation(out=ar_sb[:, W320:WID], in_=D10,
                                         func=AF.Copy, scale=1.0 / 8.0)
                    # built per t-group so the s-pass matmuls for group 0
                    # start while groups 1-2 are still being built
                    A = work.tile([128, T, N, U], bf16, tag="A")
                    cexp_v = cexp_b.rearrange("q (t n) -> q t n", t=T)
                    for gg in range(3):
                        sl = slice(3 * gg, 3 * (gg + 1))
                        nc.vector.tensor_tensor(
                            out=A[:, sl], in0=WC[:, sl],
                            in1=cexp_v[:, sl].unsqueeze(3)
                            .broadcast_to([128, 3, N, U]),
                            op=AL.mult)

                pss0 = ps_s.tile([128, NU], f32, tag="pss0")
                pss1 = ps_s.tile([128, NU], f32, tag="pss1")
                pss = [pss0, pss1]
                # h-outer: the h=0 PSUM closes after 9 matmuls, so its wire
                # cast (DVE) runs concurrently with the h=1 matmuls; only
                # the short ACT cast of h=1 remains on the trigger path.
                sX = X8 if first else X2
                for h in range(2):
                    for t in range(T):
                        nc.tensor.matmul(
                            pss[h],
                            lhsT=sX[:, t, h * 128:(h + 1) * 128],
                            rhs=(W8 if first else A)[:, t].rearrange(
                                "p n u -> p (n u)"),
                            start=(t == 0), stop=(t == T - 1))
                    if h == 0:
                        nc.vector.tensor_copy(out=ar_sb[:, 0:NU],
                                              in_=pss[0])
                nc.scalar.copy(out=ar_sb[:, NU:W320], in_=pss[1])
                nc.sync.dma_start(out=ar_in, in_=ar_sb)
                if last:
                    ar_out = dram.tile([16, wid], wdt, tag=f"arout{it}")
                    nc.gpsimd.collective_compute(
                        "ReduceScatter", AL.add,
                        ins=[ar_in.opt()], outs=[ar_out.opt()],
                        replica_groups=[list(range(NCORES))])
                    P = 16
                else:
                    # Shared-output AllReduce is the fast HBM-HBM path.
                    ar_out = nc.dram_tensor(
                        f"arout{it}", [128, wid], wdt,
                        addr_space="Shared").ap()
                    nc.gpsimd.collective_compute(
                        "AllReduce", AL.add,
                        ins=[ar_in.opt()], outs=[ar_out.opt()],
                        replica_groups=[list(range(NCORES))])
                    P = 128
                # Single readback DMA: a split D-first DMA costs ~2us (128
                # tiny 20B descriptors serialize ahead of the bulk rows).
                s_ar = small.tile([P, wid], wdt, tag=f"sar{it}")
                nc.sync.dma_start(out=s_ar, in_=ar_out)
                if first:
                    Dinv = dinv0
                else:
                    Dinv = small.tile([P, N], f32, tag=f"dinv{it}")
                    nc.vector.reciprocal(out=Dinv, in_=s_ar[:, W320:WID])
                # s_sc/sq/mag in fp16: |s_sc| <= ~0.6, mag <= ~0.35, so fp16
                # is safe and the dense sq/mag ops run in the 2x DVE mode.
                s_sc = work.tile([P, 2, N, U], f16, tag=f"ssc{it}")
                nc.vector.tensor_tensor(
                    out=s_sc,
                    in0=s_ar[:, 0:W320].rearrange(
                        "q (h n u) -> q h n u", h=2, n=N),
                    in1=Dinv.unsqueeze(1).unsqueeze(3)
                    .broadcast_to([P, 2, N, U]),
                    op=AL.mult)
                sq = work.tile([P, 2, N, U], f16, tag=f"sq{it}")
                nc.vector.tensor_tensor(out=sq, in0=s_sc, in1=s_sc,
                                        op=AL.mult)
                mag = small.tile([P, 2, U], f16, tag=f"mag{it}")
                with nc.allow_low_precision(
                        reason="mag in [0.006, 0.35]; fp16 quantization is "
                               "~5e-4 relative, far inside tolerance"):
                    nc.vector.tensor_reduce(
                        out=mag, in_=sq.rearrange("q h n u -> q h u n"),
                        axis=AX.X, op=AL.add)
                # sqrt(mag) = exp(0.5*ln(mag)) -- stays on the pinned table.
                lnm = small.tile([P, 2, U], f32, tag=f"lnm{it}")
                nc.scalar.activation(out=lnm, in_=mag, func=AF.Ln)
                sqm = small.tile([P, 2, U], f32, tag=f"sqm{it}")
                nc.scalar.activation(out=sqm, in_=lnm, func=AF.Exp,
                                     scale=0.5)
                onep = small.tile([P, 2, U], f32, tag=f"onep{it}")
                nc.vector.tensor_scalar_add(out=onep, in0=mag, scalar1=1.0)
                rec = small.tile([P, 2, U], f32, tag=f"rec{it}")
                nc.vector.reciprocal(out=rec, in_=onep)
                g = small.tile([P, 2, U], f32, tag=f"g{it}")
                nc.vector.tensor_tensor(out=g, in0=sqm, in1=rec, op=AL.mult)
                if last:
                    v_f32 = work.tile([16, 2, N, U], f32, tag="vf32")
                    nc.vector.tensor_tensor(
                        out=v_f32, in0=s_sc,
                        in1=g.unsqueeze(2).broadcast_to([16, 2, N, U]),
                        op=AL.mult)
                    nc.sync.dma_start(
                        out=v_d, in_=v_f32.rearrange("q h n u -> q (h n u)"))
                    break
                vb = work.tile([128, 2, N, U], bf16, tag="vb")
                nc.vector.tensor_tensor(
                    out=vb, in0=s_sc,
                    in1=g.unsqueeze(2).broadcast_to([128, 2, N, U]),
                    op=AL.mult)

                # ------------ a-pass (all local) ------------
                z = work.tile([128, T, N, U], bf16, tag="z")
                for grp in range(3):
                    psC = ps_c.tile([128, 3 * NU], f32)
                    for tt in range(3):
                        t = grp * 3 + tt
                        nc.tensor.matmul(
                            psC[:, tt * NU:(tt + 1) * NU],
                            lhsT=XB[:, 0, t, :],
                            rhs=vb[:, 0].rearrange("p n u -> p (n u)"),
                            start=True, stop=False)
                        nc.tensor.matmul(
                            psC[:, tt * NU:(tt + 1) * NU],
                            lhsT=XB[:, 1, t, :],
                            rhs=vb[:, 1].rearrange("p n u -> p (n u)"),
                            start=False, stop=True)
                    # z = Wc .* C with C read straight from PSUM (no copy);
                    # the u-fold tree for this group runs while the PE works
                    # on the next group's C matmuls.
                    sl = slice(grp * 3, (grp + 1) * 3)
                    nc.vector.tensor_tensor(
                        out=z[:, sl].rearrange("p t n u -> p (t n u)"),
                        in0=WC[:, sl].rearrange("p t n u -> p (t n u)"),
                        in1=psC, op=AL.mult)
                    if grp == 0:
                        t8 = small.tile([128, T, N, 8], bf16, tag="t8")
                        t4 = small.tile([128, T, N, 4], bf16, tag="t4")
                        t2 = small.tile([128, T, N, 2], bf16, tag="t2")
                        z2 = small.tile([128, T * N], bf16, tag="z2")
                        z2v = z2.rearrange("p (t n) -> p t n", t=T)
                    nc.vector.tensor_tensor(out=t8[:, sl],
                                            in0=z[:, sl, :, 0:8],
                                            in1=z[:, sl, :, 8:16], op=AL.add)
                    nc.vector.tensor_tensor(out=t4[:, sl],
                                            in0=t8[:, sl, :, 0:4],
                                            in1=t8[:, sl, :, 4:8], op=AL.add)
                    nc.vector.tensor_tensor(out=t2[:, sl],
                                            in0=t4[:, sl, :, 0:2],
                                            in1=t4[:, sl, :, 2:4], op=AL.add)
                    nc.vector.tensor_tensor(out=z2v[:, sl],
                                            in0=t2[:, sl, :, 0],
                                            in1=t2[:, sl, :, 1], op=AL.add)
                    # i-fold (sum 8-partition groups) per group, so only the
                    # last group's fold remains after the final z-mult
                    if grp == 0:
                        psA = ps_m.tile([128, 96], f32, tag="psm")
                    nc.tensor.matmul(
                        psA[0:16, 3 * grp * N:3 * (grp + 1) * N],
                        lhsT=EM, rhs=z2v[:, sl], start=True, stop=True)
                # i-broadcast via a tiny ones-matmul; then b += a/B.
                aC = small.tile([16, T * N], bf16, tag="aC")
                nc.scalar.copy(out=aC, in_=psA[0:16, 0:T * N])
                psR = ps_m.tile([128, 96], f32, tag="psm")
                nc.tensor.matmul(psR[:, 0:T * N], lhsT=EB, rhs=aC,
                                 start=True, stop=True)
                nc.vector.scalar_tensor_tensor(
                    out=brep, in0=psR[:, 0:T * N], scalar=1.0 / B, in1=brep,
                    op0=AL.mult, op1=AL.add)

    nc.compile()
    return nc


def _prep_inputs(x_full, W):
    """Host-side relayout. x_full: [B, I, J] f32, W: [J, N, U, I] f32."""
    import ml_dtypes
    bf = ml_dtypes.bfloat16
    p = np.arange(128)
    i_of_p = p % 8                       # [128]
    j16_of_p = p // 8                    # [128]
    em = (j16_of_p[:, None] == np.arange(16)[None, :]).astype(bf)   # [128,16]
    eb = np.ascontiguousarray(em.T)                                  # [16,128]
    in_maps = []
    for c in range(NCORES):
        jg = (JL * c + 16 * np.arange(T)[:, None]
              + j16_of_p[None, :])       # [T, 128] global j per (t, p)
        # Wc[p, t, n, u] = W[jg[t,p], n, u, i(p)]
        Wt = W[jg]                       # [T, 128, N, U, I]
        Wc = Wt[np.arange(T)[:, None], np.arange(128)[None, :], :, :,
                i_of_p[None, :]]         # [T, 128, N, U]
        wcf = np.ascontiguousarray(
            Wc.transpose(1, 0, 2, 3)).reshape(128, T * NU)
        wc = wcf.astype(bf)
        w8 = wcf.astype(ml_dtypes.float8_e4m3)
        # y[b, t, p] = x[b, i(p), jg[t,p]]
        y = x_full[:, i_of_p[None, :], jg]          # [B, T, 128]
        x2f = np.ascontiguousarray(
            y.transpose(2, 1, 0)).reshape(128, T * B)
        x2 = x2f.astype(bf)
        x8 = x2f.astype(ml_dtypes.float8_e4m3)
        # xb[q, h, t, m] = x[128h+q, i(m), jg[t,m]]
        xb = np.ascontiguousarray(
            y.reshape(2, 128, T, 128).transpose(1, 0, 2, 3)
        ).reshape(128, 2 * T * 128).astype(bf)
        in_maps.append({"wc": wc, "x2": x2, "w8": w8, "x8": x8,
                        "xb": xb, "em": em, "eb": eb})
    return in_maps


def _postprocess(res):
    # core c's ReduceScatter chunk = AR-buffer rows 16c..16c+16; row q,
    # col (h, n, u) is batch 128h + 16c + q.
    v = np.empty((2, 128, N, U), dtype=np.float32)
    for c in range(NCORES):
        vc = np.asarray(res.results[c]["v"], dtype=np.float32)  # [16, 320]
        v[:, 16 * c:16 * (c + 1)] = (
            vc.reshape(16, 2, N, U).transpose(1, 0, 2, 3))
    return np.ascontiguousarray(v.reshape(B, N, U)[..., None])


def kernel(x, W):
    """x: [256, 8, 1152] f32; W: [1152, 10, 16, 8] f32 ->
    v: [256, 10, 16, 1] f32."""
    from concourse.bass_utils import run_bass_kernel_spmd

    x = np.asarray(x, dtype=np.float32)
    W = np.asarray(W, dtype=np.float32)
    if "nc" not in _CACHE:
        _CACHE["nc"] = _build_nc()
    nc = _CACHE["nc"]
    in_maps = _prep_inputs(x, W)
    res = run_bass_kernel_spmd(nc, in_maps, core_ids=list(range(NCORES)))
    return _postprocess(res)


if __name__ == "__main__":
    rng = np.random.default_rng(0)
    x = rng.standard_normal((B, I, J), dtype=np.float32)
    W = rng.standard_normal((J, N, U, I), dtype=np.float32)
    got = kernel(x, W)
    # numpy reference for a self-contained smoke test
    u_hat = np.einsum('jnui,bij->bjnu', W, x)
    b = np.zeros((J, N), dtype=np.float32)
    for _ in range(ITERS):
        e = np.exp(b - b.max(axis=0, keepdims=True))
        c = e / e.sum(axis=0, keepdims=True)
        s = np.einsum('jn,bjnu->bnu', c, u_hat)
        mag = np.sum(s * s, axis=1, keepdims=True)
        v = (mag / (1.0 + mag)) * (s / np.sqrt(mag))
        b = b + np.einsum('bjnu,bnu->jn', u_hat, v) / B
    exp = v[..., None]
    rel = np.linalg.norm(got - exp) / np.linalg.norm(exp)
    print("rel_fro:", rel)
